# revision 32
# baseline (speedup 1.0000x reference)
"""Trainium2 Bass kernel v3.3 for nn_MessagePassing (GNN last-writer message passing).

Math (from the reference):
  src[l,j]  = max{ i : adj[l,i,j]==1 } U {j}          (last writer wins)
  deg[l,i]  = 1 + sum_j adj[l,i,j]
  out[j,l,:] = (W @ feature[src[l,j], l, :]) / sqrt(deg[l,src]*deg[l,j])

v3.3 strategy (vs v2 baseline, 219 us):
  - Degrees per SOURCE shard: each core popcounts the FULL 4096-bit rows of
    its own 512 sources (pbr image) -> exact deg, no partial-sum AllReduce.
    rsd = 1/sqrt(deg) AllGathered per layer half (8 KB in, 64 KB out).
  - src-finding FIRST (u16 keys: is_ne + iota-mult + reduce_max); selected
    word extracted with broadcast one-hot + AND + log2 OR-folds.
  - ALL gathers are batched dma_gather calls (indices wrapped [16, n/16]
    int16 and replicated across the 8 Q7 cores via a DRAM bounce):
    features 4096 x 512 B in ONE call; rsd as 256-B blocks per layer half
    with on-chip iota-mask extraction.
  - SWAR popcount gated on the gather indices (per-partition AP scalar dep)
    so the Tile list-scheduler cannot steal big SWAR ops into the latency
    gaps of the src chain, which would delay the gathers by ~40 us.
  - PSUM<->SBUF copies and final scaling on the Scalar (ACT) engine.
"""

import os
import sys
import types
from contextlib import ExitStack

import numpy as np

for _p in ("/opt/trn_rl_repo",):
    if os.path.isdir(_p) and _p not in sys.path:
        sys.path.append(_p)

from concourse import bacc, bass, mybir
from concourse.masks import make_identity
from concourse.tile import TileContext

F32 = mybir.dt.float32
I32 = mybir.dt.int32
I16 = mybir.dt.int16
U8 = mybir.dt.uint8
U16 = mybir.dt.uint16
AX = mybir.AxisListType
OP = mybir.AluOpType
ACT = mybir.ActivationFunctionType

P = 128  # SBUF partitions

N_NODES, N_LAYERS, D, N_CORES = 4096, 8, 128, 8


def _install_ntff_hook():
    """This image's antenv lacks axon_hooks; synthesize it so trace=True works."""
    try:
        import antenv
    except ImportError:
        return
    if "antenv.axon_hooks" in sys.modules:
        return
    mod = types.ModuleType("antenv.axon_hooks")
    _state = {"hook": None}
    mod.set_axon_ntff_profile_hook = lambda h: _state.__setitem__("hook", h)
    mod.get_axon_ntff_profile_hook = lambda: _state["hook"]
    sys.modules["antenv.axon_hooks"] = mod
    antenv.axon_hooks = mod
    try:
        from trn_agent_boot.trn_boot import _ntff_profile_via_ctypes

        mod.set_axon_ntff_profile_hook(
            _ntff_profile_via_ctypes("/opt/axon/libaxon_pjrt.so")
        )
    except Exception:
        pass


def build_kernel(N=N_NODES, L=N_LAYERS, n_cores=N_CORES, debug=False):
    """One SPMD program, identical on all cores; per-core data arrives via inputs.

    During the build, the Tile list-scheduler's SWDGE per-descriptor cost is
    calibrated to the measured dma_gather rate (~8 ns/descriptor vs the stock
    0.34 ns bulk-DMA figure). The stock value makes the scheduler believe
    gathers finish ~7x earlier than they do, so it commits gather-dependent
    PSUM copies ahead of independent work in the in-order engine streams,
    which stalls every engine behind head-of-line waits at runtime. Restored
    after the build; affects schedule quality only, not correctness.
    """
    from concourse import hw_specs

    _swdge_stock = hw_specs.TRN2Spec.SWDGE_NS_PER_DESCRIPTOR
    hw_specs.TRN2Spec.SWDGE_NS_PER_DESCRIPTOR = 8.0
    JJ = N // n_cores  # dests (= own sources) per core
    T = JJ // P  # 128-row tiles per layer
    G = L * T  # (layer, tile) groups
    GH = G // 2  # groups per layer half
    LH = L // 2
    WR = N // 32  # i32 words per 4096-bit row
    GW = G * WR  # i32 words per partition per image
    HT = JJ * LH  # rsd entries per core per layer half
    NI = P * G  # feature gather count
    NIH = P * GH  # rsd gather count per half
    DB = 64  # f32 per gathered rsd block (256 B)

    nc = bacc.Bacc()
    pbt = nc.declare_dram_parameter("pbt", [P, GW], I32, isOutput=False)
    pbr = nc.declare_dram_parameter("pbr", [P, GW], I32, isOutput=False)
    featg = nc.declare_dram_parameter("featg", [N * L, D], F32, isOutput=False)
    wt = nc.declare_dram_parameter("wt", [D, D], F32, isOutput=False)
    iotw = nc.declare_dram_parameter("iotw", [P, WR], U16, isOutput=False)
    iotwf = nc.declare_dram_parameter("iotwf", [P, WR], F32, isOutput=False)
    iodbf = nc.declare_dram_parameter("iodbf", [P, DB], F32, isOutput=False)
    jg = nc.declare_dram_parameter("jg", [P, G], F32, isOutput=False)
    li32 = nc.declare_dram_parameter("li32", [P, G], I32, isOutput=False)
    lh4 = nc.declare_dram_parameter("lh4", [P, G], I32, isOutput=False)
    lf32 = nc.declare_dram_parameter("lf32", [P, G], F32, isOutput=False)
    bmat = nc.declare_dram_parameter("bmat", [16, P], F32, isOutput=False)
    out = nc.declare_dram_parameter("out", [JJ, L, D], F32, isOutput=True)

    if debug:
        o_src = nc.declare_dram_parameter("o_src", [P, G], F32, isOutput=True)
        o_deg = nc.declare_dram_parameter("o_deg", [P, G], F32, isOutput=True)
        o_rsds = nc.declare_dram_parameter("o_rsds", [P, G], F32, isOutput=True)
        o_comb = nc.declare_dram_parameter("o_comb", [P, G], F32, isOutput=True)

    # warmup scratch for the first-collective setup cost
    cwarm_in = nc.dram_tensor("cwarm_in", [16], F32)
    cwarm_out = nc.dram_tensor("cwarm_out", [16 * n_cores], F32, addr_space="Shared")
    # rsd table layout per core per half: [p, (l%4), t] (contiguous writes)
    rsd_in = nc.dram_tensor("rsd_in", [L * JJ], F32)
    rsd_all1 = nc.dram_tensor("rsd_all1", [n_cores * HT], F32, addr_space="Shared")
    rsd_all2 = nc.dram_tensor("rsd_all2", [n_cores * HT], F32, addr_space="Shared")

    with TileContext(nc) as tc, ExitStack() as ctx:
        const = ctx.enter_context(tc.tile_pool(name="const", bufs=1))
        big = ctx.enter_context(tc.tile_pool(name="big", bufs=1))
        sml = ctx.enter_context(tc.tile_pool(name="sml", bufs=1))
        psum = ctx.enter_context(tc.tile_pool(name="psum", bufs=2, space="PSUM"))
        psum2 = ctx.enter_context(tc.tile_pool(name="psum2", bufs=2, space="PSUM"))
        psumw = ctx.enter_context(tc.tile_pool(name="psumw", bufs=1, space="PSUM"))

        # ---- input DMAs (pbt first: it gates src-finding, the critical path) ----
        pbt_sb = big.tile([P, GW], I32, tag="pbt")
        nc.sync.dma_start(pbt_sb[:, 0 : GW // 2], pbt.ap()[:, 0 : GW // 2])
        nc.sync.dma_start(pbt_sb[:, GW // 2 : GW], pbt.ap()[:, GW // 2 : GW])
        pbr_sb = big.tile([P, GW], I32, tag="pbr")
        nc.sync.dma_start(pbr_sb[:, 0 : GW // 2], pbr.ap()[:, 0 : GW // 2])
        nc.sync.dma_start(pbr_sb[:, GW // 2 : GW], pbr.ap()[:, GW // 2 : GW])

        iotw_sb = const.tile([P, WR], U16)
        nc.sync.dma_start(iotw_sb[:], iotw.ap())
        iotwf_sb = const.tile([P, WR], F32)
        nc.sync.dma_start(iotwf_sb[:], iotwf.ap())
        iodbf_sb = const.tile([P, DB], F32)
        nc.sync.dma_start(iodbf_sb[:], iodbf.ap())
        wt_sb = const.tile([D, D], F32)
        nc.sync.dma_start(wt_sb[:], wt[:])
        jg_sb = const.tile([P, G], F32)
        nc.sync.dma_start(jg_sb[:], jg.ap())
        li32_sb = const.tile([P, G], I32)
        nc.sync.dma_start(li32_sb[:], li32.ap())
        lh4_sb = const.tile([P, G], I32)
        nc.sync.dma_start(lh4_sb[:], lh4.ap())
        lf32_sb = const.tile([P, G], F32)
        nc.sync.dma_start(lf32_sb[:], lf32.ap())
        bmat_sb = const.tile([16, P], F32)
        nc.sync.dma_start(bmat_sb[:], bmat.ap())
        eye = const.tile([P, P], F32)


        # gather output tiles, memzeroed first on the idle Pool engine
        featsb = big.tile([P, G * D], F32, tag="featsb")
        rblk = big.tile([P, G * DB], F32, tag="rblk")
        nc.gpsimd.memset(featsb[:], 0.0)
        nc.gpsimd.memset(rblk[:], 0.0)
        make_identity(nc, eye[:])
        # fire a tiny collective immediately: absorbs the first-use setup
        # cost so the real AllGathers run at steady-state latency
        nc.gpsimd.collective_compute(
            "AllGather",
            OP.bypass,
            ins=[cwarm_in.ap()],
            outs=[cwarm_out.ap()],
            replica_groups=[list(range(n_cores))],
        )

        # ---- phase S1: last-nonzero-word keys (Vector) ----
        s1 = big.tile([P, GW], I32, tag="s1")
        s2 = big.tile([P, GW], I32, tag="s2")
        s3 = big.tile([P, GW], I32, tag="s3")
        ind = s1[:].bitcast(U16)[:, 0:GW]
        key = s2[:].bitcast(U16)[:, 0:GW]
        nc.vector.tensor_scalar(ind, pbt_sb[:], 0, None, OP.not_equal)
        nc.vector.tensor_tensor(
            key.rearrange("p (g w) -> p g w", w=WR),
            ind.rearrange("p (g w) -> p g w", w=WR),
            iotw_sb[:].rearrange("p (u w) -> p u w", u=1).to_broadcast([P, G, WR]),
            op=OP.mult,
        )
        wp1 = sml.tile([P, G], U16, tag="wp1")
        nc.vector.tensor_reduce(
            wp1[:], key.rearrange("p (g w) -> p g w", w=WR), axis=AX.X, op=OP.max
        )

        # ---- phase S2: extract the selected word (one-hot AND + OR-folds) ----
        wp1f = sml.tile([P, G], F32, tag="wp1f")
        nc.vector.tensor_copy(wp1f[:], wp1[:])
        nc.vector.tensor_tensor(
            s1[:].rearrange("p (g w) -> p g w", w=WR),
            iotwf_sb[:].rearrange("p (u w) -> p u w", u=1).to_broadcast([P, G, WR]),
            wp1f[:].rearrange("p (g u) -> p g u", u=1).to_broadcast([P, G, WR]),
            op=OP.is_equal,
        )
        nc.vector.tensor_scalar(
            s1[:], s1[:], 31, 31, OP.logical_shift_left, OP.arith_shift_right
        )
        nc.vector.tensor_tensor(s2[:], pbt_sb[:], s1[:], op=OP.bitwise_and)
        sv = s2[:].rearrange("p (g w) -> p g w", w=WR)
        tv = s3[:].rearrange("p (g w) -> p g w", w=WR)
        nc.vector.tensor_tensor(tv[:, :, 0:64], sv[:, :, 0:64], sv[:, :, 64:128], op=OP.bitwise_or)
        nc.vector.tensor_tensor(tv[:, :, 64:96], tv[:, :, 0:32], tv[:, :, 32:64], op=OP.bitwise_or)
        nc.vector.tensor_tensor(tv[:, :, 96:112], tv[:, :, 64:80], tv[:, :, 80:96], op=OP.bitwise_or)
        nc.vector.tensor_tensor(tv[:, :, 112:120], tv[:, :, 96:104], tv[:, :, 104:112], op=OP.bitwise_or)
        nc.vector.tensor_tensor(tv[:, :, 120:124], tv[:, :, 112:116], tv[:, :, 116:120], op=OP.bitwise_or)
        nc.vector.tensor_tensor(tv[:, :, 124:126], tv[:, :, 120:122], tv[:, :, 122:124], op=OP.bitwise_or)
        sel = sml.tile([P, G], I32, tag="sel")
        nc.vector.tensor_tensor(
            sel[:].rearrange("p (g u) -> p g u", u=1),
            tv[:, :, 124:125],
            tv[:, :, 125:126],
            op=OP.bitwise_or,
        )

        # ---- phase S3: msb of selected word (u16 halves + f32-exponent) ----
        lo = sml.tile([P, G], I32, tag="lo")
        hi = sml.tile([P, G], I32, tag="hi")
        lof = sml.tile([P, G], F32, tag="lof")
        hif = sml.tile([P, G], F32, tag="hif")
        nc.vector.tensor_scalar(lo[:], sel[:], 0xFFFF, None, OP.bitwise_and)
        nc.vector.tensor_scalar(hi[:], sel[:], 16, None, OP.logical_shift_right)
        nc.vector.tensor_copy(lof[:], lo[:])
        nc.vector.tensor_copy(hif[:], hi[:])
        loe = sml.tile([P, G], I32, tag="loe")
        hie = sml.tile([P, G], I32, tag="hie")
        nc.vector.tensor_scalar(
            loe[:], lof[:].bitcast(I32), 23, None, OP.logical_shift_right
        )
        nc.vector.tensor_scalar(
            hie[:], hif[:].bitcast(I32), 23, None, OP.logical_shift_right
        )
        nc.vector.tensor_scalar(hie[:], hie[:], 16, None, OP.add)
        bp = sml.tile([P, G], I32, tag="bp")
        nc.vector.tensor_tensor(bp[:], hie[:], loe[:], op=OP.max)
        bpf = sml.tile([P, G], F32, tag="bpf")
        nc.vector.tensor_copy(bpf[:], bp[:])
        sa = sml.tile([P, G], F32, tag="sa")
        nc.vector.tensor_scalar(sa[:], wp1f[:], 32.0, -159.0, OP.mult, OP.add)
        srcf = sml.tile([P, G], F32, tag="srcf")
        nc.vector.tensor_tensor(srcf[:], sa[:], bpf[:], op=OP.add)
        src = sml.tile([P, G], F32, tag="src")
        nc.vector.tensor_tensor(src[:], srcf[:], jg_sb[:], op=OP.max)

        # gather indices: fidx = src*L + l (features);
        # ridx = (src>>9)*2048 + (l%4)*512 + (src&511) (per-half rsd tables)
        srci = sml.tile([P, G], I32, tag="srci")
        nc.vector.tensor_copy(srci[:], src[:])
        fidxf = sml.tile([P, G], F32, tag="fidxf")
        nc.vector.tensor_scalar(fidxf[:], src[:], 8.0, None, OP.mult)
        nc.vector.tensor_tensor(fidxf[:], fidxf[:], lf32_sb[:], op=OP.add)
        ridx = sml.tile([P, G], I32, tag="ridx")
        rbx = sml.tile([P, G], I32, tag="rbx")
        rcx = sml.tile([P, G], I32, tag="rcx")
        nc.vector.tensor_scalar(
            ridx[:], srci[:], 9, 11, OP.logical_shift_right, OP.logical_shift_left
        )
        nc.vector.tensor_scalar(
            rbx[:], srci[:], 127, 4, OP.bitwise_and, OP.logical_shift_left
        )
        nc.vector.tensor_scalar(
            rcx[:], srci[:], 7, 3, OP.logical_shift_right, OP.bitwise_and
        )
        nc.vector.tensor_tensor(ridx[:], ridx[:], rbx[:], op=OP.add)
        nc.vector.tensor_tensor(ridx[:], ridx[:], rcx[:], op=OP.add)
        nc.vector.tensor_tensor(ridx[:], ridx[:], lh4_sb[:], op=OP.add)
        # block index + in-block offset for the 256-B rsd gather
        rblki = sml.tile([P, G], I32, tag="rblki")
        rofs = sml.tile([P, G], I32, tag="rofs")
        nc.vector.tensor_scalar(rblki[:], ridx[:], 6, None, OP.logical_shift_right)
        nc.vector.tensor_scalar(rofs[:], ridx[:], 63, None, OP.bitwise_and)
        rofsf = sml.tile([P, G], F32, tag="rofsf")
        nc.vector.tensor_copy(rofsf[:], rofs[:])
        rblkif = sml.tile([P, G], F32, tag="rblkif")
        nc.vector.tensor_copy(rblkif[:], rblki[:])

        # ---- build wrapped-replicated int16 index tiles ON-CHIP ----
        # wrapped[q, g*8+a] = idx[a*16+q, g]; then replicate the 16 rows to
        # all 128 partitions with a 0/1 broadcast matmul. Exact in f32.
        def pe_wrap_build(idxf, tag):
            ptr = psumw.tile([P, P], F32, tag="wrT")
            nc.tensor.transpose(ptr[0:G, :], idxf[:, :], eye[:])
            ft = sml.tile([G, P], F32, tag=tag + "_ft")
            nc.scalar.copy(ft[:], ptr[0:G, :])
            wf = sml.tile([16, 8 * G], F32, tag=tag + "_wf")
            for a in range(8):
                pa = psumw.tile([16, G], F32, tag="wrA")
                nc.tensor.transpose(pa[:], ft[:, 16 * a : 16 * (a + 1)], eye[0:G, 0:G])
                nc.scalar.copy(
                    wf[:].rearrange("q (g a) -> q g a", a=8)[:, :, a], pa[:]
                )
            pb = psumw.tile([P, 8 * G], F32, tag=tag + "_wrB")
            nc.tensor.matmul(pb[:], lhsT=bmat_sb[:], rhs=wf[:], start=True, stop=True)
            return pb

        def pe_wrap_convert(pb, tag):
            w16 = sml.tile([P, 8 * G], I16, tag=tag + "_w16")
            nc.vector.tensor_copy(w16[:], pb[:])
            return w16

        with tc.high_priority():
            fw = pe_wrap_convert(pe_wrap_build(fidxf[:], "fw"), "fw")
            rw_pb = pe_wrap_build(rblkif[:], "rw")

        # gate the SWAR on the gather-index chain: AND one pbr word per half
        # with 0xFFFFFFFF (no-op data-wise) so the big SWAR ops cannot be
        # list-scheduled into the src chain's latency gaps.
        g1 = sml.tile([P, 1], I32, tag="g1")
        gFF = sml.tile([P, 1], I32, tag="gFF")
        nc.vector.tensor_scalar(g1[:], rofsf[:, 0:1], 0.0, None, OP.is_ge)
        nc.vector.tensor_scalar(
            gFF[:], g1[:], 31, 31, OP.logical_shift_left, OP.arith_shift_right
        )
        for h in range(2):
            c0 = h * (GW // 2)
            nc.vector.tensor_scalar(
                pbr_sb[:, c0 : c0 + 1],
                pbr_sb[:, c0 : c0 + 1],
                gFF[:, 0:1],
                None,
                OP.bitwise_and,
            )

        # ---- ONE batched feature gather: 4096 rows x 512 B (Pool) ----
        NQ = NI // 4  # 1024 indices (8 groups) per chunk
        for q in range(4):
            nc.gpsimd.dma_gather(
                out_ap=featsb[:, q * 8 * D : (q + 1) * 8 * D].rearrange(
                    "p (g d) -> p g d", d=D
                ),
                in_ap=featg.ap(),
                idxs_ap=fw[:, q * (NQ // 16) : (q + 1) * (NQ // 16)],
                num_idxs=NQ,
                num_idxs_reg=NQ,
                elem_size=D,
                single_packet=False,
            )

        # ---- feature transposes + W matmuls (PE; copies on ACT) ----
        gts = big.tile([P, G * P], F32, tag="gts")
        stage = big.tile([P, G * D], F32, tag="stage")
        for g in range(G):
            pt = psum.tile([P, P], F32, tag="pt")
            nc.tensor.transpose(pt[:], featsb[:, g * D : (g + 1) * D], eye[:])
            nc.scalar.copy(gts[:, g * P : (g + 1) * P], pt[:])
        for g in range(G):
            po = psum2.tile([P, P], F32, tag="po")
            nc.tensor.matmul(
                po[:],
                lhsT=gts[:, g * P : (g + 1) * P],
                rhs=wt_sb[:],
                start=True,
                stop=True,
            )
            nc.scalar.copy(stage[:, g * D : (g + 1) * D], po[:])

        # ---- SWAR popcount of pbr rows, one layer half at a time (Vector) ----
        degf = sml.tile([P, G], F32, tag="degf")
        rsd = sml.tile([P, G], F32, tag="rsd")
        xx = sml.tile([P, G], F32, tag="xx")
        ihi = sml.tile([P, G], I32, tag="ihi")
        ilo = sml.tile([P, G], I32, tag="ilo")
        av = sml.tile([P, G], I32, tag="av")
        ylo = sml.tile([P, G], I32, tag="ylo")
        yb = sml.tile([P, G], I32, tag="yb")
        t1 = sml.tile([P, G], F32, tag="t1q")
        for h in range(2):
            u = slice(2 * h * GH * WR, 2 * (h + 1) * GH * WR)  # u16 cols
            gsl = slice(h * GH, (h + 1) * GH)
            vu = pbr_sb[:].bitcast(U16)[:, u]
            t1u = s1[:].bitcast(U16)[:, u]
            t2u = s2[:].bitcast(U16)[:, u]
            t3u = s3[:].bitcast(U16)[:, u]
            nc.vector.tensor_scalar(
                t1u, vu, 1, 0x5555, OP.logical_shift_right, OP.bitwise_and
            )
            nc.vector.tensor_tensor(t2u, vu, t1u, op=OP.subtract)
            nc.vector.tensor_scalar(t1u, t2u, 0x3333, None, OP.bitwise_and)
            nc.vector.tensor_scalar(
                t3u, t2u, 2, 0x3333, OP.logical_shift_right, OP.bitwise_and
            )
            nc.vector.tensor_tensor(t2u, t1u, t3u, op=OP.add)
            nc.vector.tensor_scalar(t1u, t2u, 4, None, OP.logical_shift_right)
            nc.vector.tensor_tensor(t2u, t2u, t1u, op=OP.add)
            nc.vector.tensor_scalar(t1u, t2u, 0x0F0F, None, OP.bitwise_and)
            # byte counts (<=8) in u16 lanes; fold 256 -> 32, merge bytes, reduce
            bs = s1[:].bitcast(U16).rearrange("p (g w) -> p g w", w=2 * WR)[:, gsl]
            f1 = s2[:].bitcast(U16).rearrange("p (g w) -> p g w", w=2 * WR)[:, gsl]
            f2 = s3[:].bitcast(U16).rearrange("p (g w) -> p g w", w=2 * WR)[:, gsl]
            with nc.allow_low_precision(reason="exact small-int popcount"):
                nc.vector.tensor_tensor(
                    f1[:, :, 0:128], bs[:, :, 0:128], bs[:, :, 128:256], op=OP.add
                )
                nc.vector.tensor_tensor(
                    f1[:, :, 128:192], f1[:, :, 0:64], f1[:, :, 64:128], op=OP.add
                )
                nc.vector.tensor_tensor(
                    f1[:, :, 192:224], f1[:, :, 128:160], f1[:, :, 160:192], op=OP.add
                )
                nc.vector.tensor_scalar(
                    f2[:, :, 0:32], f1[:, :, 192:224], 0x00FF, None, OP.bitwise_and
                )
                nc.vector.tensor_scalar(
                    f2[:, :, 32:64], f1[:, :, 192:224], 8, None, OP.logical_shift_right
                )
                nc.vector.tensor_tensor(
                    f2[:, :, 64:96], f2[:, :, 0:32], f2[:, :, 32:64], op=OP.add
                )
                nc.vector.tensor_reduce(
                    degf[:, gsl], f2[:, :, 64:96], axis=AX.X, op=OP.add
                )
            # rsd = 1/sqrt(deg+1) on Vector only (ACT sits behind the PSUM
            # copies that block on the feature gather). Quake magic seed,
            # built with exact u16-lane borrow arithmetic (i32 subtract would
            # round through the fp32 upcast), + one Newton step (~0.18% err).
            gx = gsl
            nc.vector.tensor_scalar(xx[:, gx], degf[:, gx], 1.0, None, OP.add)
            xb = xx[:].bitcast(I32)
            nc.vector.tensor_scalar(ihi[:, gx], xb[:, gx], 17, None, OP.logical_shift_right)
            nc.vector.tensor_scalar(
                ilo[:, gx], xb[:, gx], 1, 0xFFFF, OP.logical_shift_right, OP.bitwise_and
            )
            nc.vector.tensor_scalar(ilo[:, gx], ilo[:, gx], 0xFFFF, None, OP.bitwise_xor)
            nc.vector.tensor_scalar(av[:, gx], ilo[:, gx], 0x59E0, None, OP.add)
            nc.vector.tensor_scalar(ylo[:, gx], av[:, gx], 0xFFFF, None, OP.bitwise_and)
            nc.vector.tensor_scalar(av[:, gx], av[:, gx], 16, None, OP.logical_shift_right)
            nc.vector.tensor_scalar(ihi[:, gx], ihi[:, gx], 0xFFFF, None, OP.bitwise_xor)
            nc.vector.tensor_scalar(ihi[:, gx], ihi[:, gx], 0x5F37, None, OP.add)
            nc.vector.tensor_tensor(ihi[:, gx], ihi[:, gx], av[:, gx], op=OP.add)
            nc.vector.tensor_scalar(ihi[:, gx], ihi[:, gx], 0xFFFF, None, OP.bitwise_and)
            nc.vector.tensor_scalar(ihi[:, gx], ihi[:, gx], 16, None, OP.logical_shift_left)
            nc.vector.tensor_tensor(yb[:, gx], ihi[:, gx], ylo[:, gx], op=OP.bitwise_or)
            yf = yb[:].bitcast(F32)
            nc.vector.tensor_tensor(t1[:, gx], yf[:, gx], yf[:, gx], op=OP.mult)
            nc.vector.tensor_tensor(t1[:, gx], t1[:, gx], xx[:, gx], op=OP.mult)
            nc.vector.tensor_scalar(t1[:, gx], t1[:, gx], -0.5, 1.5, OP.mult, OP.add)
            nc.vector.tensor_tensor(rsd[:, gx], yf[:, gx], t1[:, gx], op=OP.mult)
            nc.sync.dma_start(
                rsd_in.ap()
                .rearrange("(h p k) -> h p k", h=2, p=P)[h],
                rsd[:, gsl],
            )
            with tc.high_priority():
                nc.gpsimd.collective_compute(
                    "AllGather",
                    OP.bypass,
                    ins=[rsd_in.ap()[h * HT : (h + 1) * HT]],
                    outs=[(rsd_all1 if h == 0 else rsd_all2).ap()],
                    replica_groups=[list(range(n_cores))],
                )

        # the rw convert is emitted after the SWAR so the Vector CAST cannot
        # be committed into the SWAR's latency gaps (the PE/ACT part ran early)
        rw = pe_wrap_convert(rw_pb, "rw")

        # ---- rsd-src gathers: one 256-B-block dma_gather per half (Pool) ----
        rsds = sml.tile([P, G], F32, tag="rsds")
        comb = sml.tile([P, G], F32, tag="comb")
        for h in range(2):
            gsl = slice(h * GH, (h + 1) * GH)
            tab = (rsd_all1 if h == 0 else rsd_all2).ap().rearrange(
                "(r c) -> r c", c=DB
            )
            nc.gpsimd.dma_gather(
                out_ap=rblk[:, h * GH * DB : (h + 1) * GH * DB].rearrange(
                    "p (g d) -> p g d", d=DB
                ),
                in_ap=tab,
                idxs_ap=rw[:, h * (NIH // 16) : (h + 1) * (NIH // 16)],
                num_idxs=NIH,
                num_idxs_reg=NIH,
                elem_size=DB,
                single_packet=False,
            )
            # extract rsd value at offset (ridx & 63) from each block (Pool, f32)
            bm = s1[:].bitcast(F32)[:, h * GH * DB : (h + 1) * GH * DB].rearrange(
                "p (g d) -> p g d", d=DB
            )
            bv = s2[:].bitcast(F32)[:, h * GH * DB : (h + 1) * GH * DB].rearrange(
                "p (g d) -> p g d", d=DB
            )
            rbv = rblk[:, h * GH * DB : (h + 1) * GH * DB].rearrange(
                "p (g d) -> p g d", d=DB
            )
            nc.vector.tensor_tensor(
                bm,
                iodbf_sb[:].rearrange("p (u d) -> p u d", u=1).to_broadcast([P, GH, DB]),
                rofsf[:, gsl].rearrange("p (g u) -> p g u", u=1).to_broadcast([P, GH, DB]),
                op=OP.is_equal,
            )
            nc.vector.tensor_tensor(bv, rbv, bm, op=OP.mult)
            nc.vector.tensor_tensor(bv[:, :, 0:32], bv[:, :, 0:32], bv[:, :, 32:64], op=OP.add)
            nc.vector.tensor_tensor(bv[:, :, 32:48], bv[:, :, 0:16], bv[:, :, 16:32], op=OP.add)
            nc.vector.tensor_tensor(bv[:, :, 48:56], bv[:, :, 32:40], bv[:, :, 40:48], op=OP.add)
            nc.vector.tensor_tensor(bv[:, :, 56:60], bv[:, :, 48:52], bv[:, :, 52:56], op=OP.add)
            nc.vector.tensor_tensor(bv[:, :, 60:62], bv[:, :, 56:58], bv[:, :, 58:60], op=OP.add)
            nc.vector.tensor_tensor(
                rsds[:, gsl].rearrange("p (g u) -> p g u", u=1),
                bv[:, :, 60:61],
                bv[:, :, 61:62],
                op=OP.add,
            )
            nc.vector.tensor_tensor(
                comb[:, gsl], rsds[:, gsl], rsd[:, gsl], op=OP.mult
            )
            # scale + store this half's output, one dest tile at a time
            for t in range(T):
                for l in range(h * LH, (h + 1) * LH):
                    g = l * T + t
                    nc.scalar.activation(
                        stage[:, g * D : (g + 1) * D],
                        stage[:, g * D : (g + 1) * D],
                        ACT.Copy,
                        scale=comb[:, g : g + 1],
                    )
                nc.sync.dma_start(
                    out[t * P : (t + 1) * P, h * LH : (h + 1) * LH, :],
                    stage[:].rearrange("p (l t d) -> p t l d", t=T, d=D)[
                        :, t, h * LH : (h + 1) * LH
                    ],
                )

        if debug:
            nc.sync.dma_start(o_src.ap(), src[:])
            nc.sync.dma_start(o_deg.ap(), degf[:])
            nc.sync.dma_start(o_rsds.ap(), rsds[:])
            nc.sync.dma_start(o_comb.ap(), comb[:])

    nc.finalize()
    hw_specs.TRN2Spec.SWDGE_NS_PER_DESCRIPTOR = _swdge_stock
    return nc


def shard_inputs(feature, W, adj, N=N_NODES, L=N_LAYERS, n_cores=N_CORES):
    """Host-side sharding/layout prep: bit-packing + layout transforms only."""
    JJ = N // n_cores
    T = JJ // P
    G = L * T
    WR = N // 32
    GW = G * WR
    DB = 64
    featg = np.ascontiguousarray(
        np.asarray(feature, dtype=np.float32).reshape(N * L, D)
    )
    wtr = np.ascontiguousarray(np.asarray(W, dtype=np.float32).T)
    a01 = np.asarray(adj) == 1  # [L, N(src), N(dest)] bool

    iotw = np.tile(np.arange(1, WR + 1, dtype=np.uint16), (P, 1))
    iotwf = iotw.astype(np.float32)
    iodbf = np.tile(np.arange(DB, dtype=np.float32), (P, 1))
    gl = np.repeat(np.arange(L), T)  # l per group
    gtt = np.tile(np.arange(T), L)  # t per group
    pp = np.arange(P)[:, None]
    li32 = np.tile(gl, (P, 1)).astype(np.int32)
    lh4 = ((li32 % (L // 2)) * 4).astype(np.int32)
    lf32 = li32.astype(np.float32)
    bmat = (np.arange(P)[None, :] % 16 == np.arange(16)[:, None]).astype(np.float32)
    common = {
        "featg": featg,
        "wt": wtr,
        "iotw": iotw,
        "iotwf": iotwf,
        "iodbf": iodbf,
        "li32": li32,
        "lh4": lh4,
        "lf32": lf32,
        "bmat": bmat,
    }

    in_maps = []
    for c in range(n_cores):
        j0 = c * JJ
        # pbt: rows (l, t, p) = own dest j0+t*128+p; bits over source i
        bt = np.packbits(
            a01[:, :, j0 : j0 + JJ].transpose(0, 2, 1), axis=-1, bitorder="little"
        )
        pbt = bt.reshape(L, T, P, WR * 4).transpose(2, 0, 1, 3).reshape(P, GW * 4)
        pbt = np.ascontiguousarray(pbt).view(np.int32)
        # pbr: rows (l, t, p) = own source j0+t*128+p; bits over ALL dests j
        br = np.packbits(a01[:, j0 : j0 + JJ, :], axis=-1, bitorder="little")
        pbr = br.reshape(L, T, P, WR * 4).transpose(2, 0, 1, 3).reshape(P, GW * 4)
        pbr = np.ascontiguousarray(pbr).view(np.int32)
        jgv = (j0 + gtt[None, :] * P + pp).astype(np.float32)
        in_maps.append({"pbt": pbt, "pbr": pbr, "jg": jgv, **common})
    return in_maps


_NC_CACHE = {}
LAST_RESULT = None


def kernel(feature, W, adj):
    global LAST_RESULT
    _install_ntff_hook()
    from concourse.bass_utils import run_bass_kernel_spmd

    feature = np.asarray(feature)
    W = np.asarray(W)
    adj = np.asarray(adj)
    N, L, _ = feature.shape
    key = (N, L)
    if key not in _NC_CACHE:
        _NC_CACHE[key] = build_kernel(N=N, L=L)
    nc = _NC_CACHE[key]

    in_maps = shard_inputs(feature, W, adj, N=N, L=L)
    res = run_bass_kernel_spmd(nc, in_maps, core_ids=list(range(N_CORES)))
    LAST_RESULT = res
    return np.concatenate([res.results[c]["out"] for c in range(N_CORES)], axis=0)
